# revision 5
# baseline (speedup 1.0000x reference)
"""HSIC loss kernel for Trainium2, SPMD over 8 NeuronCores.

Math (reference): K = exp(-d2(x)), L = exp(-d2(y)),
  hsic = (sum(L*K) - 2*dot(rK,rL)/m + sum(K)*sum(L)/m^2) / (m-1)^2
where rK_i = sum_j K_ij (row sums; K, L symmetric).

Sharding: rows of the Gram matrices are split into 8 strips of 1024.
Each core receives ONLY its own strip of x and y (transposed, bf16)
plus tiny per-row metadata; the full x^T/y^T moving operands are
assembled on-device with an AllGather collective. This keeps the
host->device wire traffic at ~5 MB/call (vs ~39 MB if every core's
full rotated copy were shipped from the host), which dominates the
end-to-end latency on the axon-tunneled PJRT transport.

Per core, the [1024, 8192] strips of K and L are computed fully fused
(never materialized in DRAM):
  PSUM = x_strip @ x_full^T  (bf16 matmul, D=128 contraction)
         + rank-2 correction folding in -sq_j/2 (bf16 hi/lo split)
  K    = ACT exp(2*PSUM - sq_i)  (per-partition bias, scale=2)
The diagonal needs exact treatment (off-diagonal entries are ~e-30;
the diagonal K_ii = 1 carries the whole answer). Because the strips
are gathered in natural order, the diagonal block position would be
core-dependent, which a static SPMD program cannot address. Instead
the main pass INCLUDES the (slightly inexact) diagonal, and a second
tiny pass recomputes the 8 diagonal [128,128] blocks bit-identically
from the local strip (same operand values, same accumulation order),
extracts their diagonals, and subtracts them from the row sums and
the K*L sum. The true diagonal (exp(0)=1) is re-added analytically
on the host - exact math, not an approximation.

Per-core output is a single [128, 17] f32 tensor: row sums of K and
L by chunk (diag excluded) and the K*L partial sum. Host combines in
float64.
"""

import numpy as np
import ml_dtypes

BF16 = ml_dtypes.bfloat16

M = 8192
D = 128
NDEV = 8
STRIP = M // NDEV          # 1024 rows per core
NCHUNK = STRIP // 128      # 8 partition chunks per strip
SUPER = 2048               # ACT/PSUM super-tile width (4 PSUM banks)
NSUP = M // SUPER          # 4 j-supers
TS = 512                   # matmul free-dim tile (one PSUM bank)

R2W = M + STRIP            # 9216: full-M correction row + own-strip slice
NSLOT = NCHUNK * NSUP      # 32 accumulation slots

_cache = {}

OPTS = {"repeat": 1}


def _build_program():
    import concourse.bacc as bacc
    import concourse.mybir as mybir
    from concourse import tile

    f32 = mybir.dt.float32
    bf16 = mybir.dt.bfloat16
    Exp = mybir.ActivationFunctionType.Exp
    mult = mybir.AluOpType.mult
    add = mybir.AluOpType.add

    nc = bacc.Bacc("TRN2", target_bir_lowering=False, debug=False,
                   num_devices=NDEV)

    # DRAM inputs (per-core values differ, same shapes: SPMD)
    xys_d = nc.dram_tensor("xys", [STRIP, 2 * D], bf16, kind="ExternalInput")
    r2_d = nc.dram_tensor("r2", [2, 2 * R2W], bf16, kind="ExternalInput")
    nsq_d = nc.dram_tensor("nsq", [128, 2 * NCHUNK], f32, kind="ExternalInput")
    eye_d = nc.dram_tensor("eye", [128, 128], bf16, kind="ExternalInput")

    out_d = nc.dram_tensor("out", [128, 17], f32, kind="ExternalOutput")

    with tile.TileContext(nc) as tc:
        with (
            tc.tile_pool(name="dram", bufs=1, space="DRAM") as dram,
            tc.tile_pool(name="const", bufs=1) as cpool,
            tc.tile_pool(name="psum", bufs=2, space="PSUM") as pspool,
            tc.tile_pool(name="kl", bufs=2) as klpool,
            tc.tile_pool(name="scr", bufs=2) as scrpool,
        ):
            # --- AllGather the x/y strips into full moving operands ---
            cc_in = dram.tile([STRIP, 2 * D], bf16)
            cc_out = dram.tile([NDEV * STRIP, 2 * D], bf16)
            nc.gpsimd.dma_start(out=cc_in[:, :], in_=xys_d[:, :])
            nc.gpsimd.collective_compute(
                "AllGather",
                mybir.AluOpType.bypass,
                replica_groups=[list(range(NDEV))],
                ins=[cc_in.opt()],
                outs=[cc_out.opt()],
            )

            xTs = cpool.tile([128, STRIP], bf16, tag="xTs")
            yTs = cpool.tile([128, STRIP], bf16, tag="yTs")
            r2x = cpool.tile([2, R2W], bf16, tag="r2x")
            r2y = cpool.tile([2, R2W], bf16, tag="r2y")
            nsq = cpool.tile([128, 2 * NCHUNK], f32, tag="nsq")
            eye = cpool.tile([128, 128], bf16, tag="eye")
            ones2 = cpool.tile([2, D], bf16, tag="ones2")
            xG = cpool.tile([128, M], bf16, tag="xG")
            yG = cpool.tile([128, M], bf16, tag="yG")
            accK = cpool.tile([128, NSLOT], f32, tag="accK")
            accL = cpool.tile([128, NSLOT], f32, tag="accL")
            accS = cpool.tile([128, NSLOT], f32, tag="accS")
            diagK = cpool.tile([128, NCHUNK], f32, tag="diagK")
            diagL = cpool.tile([128, NCHUNK], f32, tag="diagL")
            out_sb = cpool.tile([128, 17], f32, tag="out")
            t1 = cpool.tile([128, NCHUNK], f32, tag="t1")
            t2 = cpool.tile([128, NCHUNK], f32, tag="t2")
            u1 = cpool.tile([128, NCHUNK], f32, tag="u1")
            u2 = cpool.tile([128, NCHUNK], f32, tag="u2")

            nc.sync.dma_start_transpose(out=xTs[:, :], in_=xys_d[:, 0:D])
            nc.sync.dma_start_transpose(out=yTs[:, :], in_=xys_d[:, D:2 * D])
            nc.gpsimd.dma_start(out=r2x[:, :], in_=r2_d[:, 0:R2W])
            nc.gpsimd.dma_start(out=r2y[:, :], in_=r2_d[:, R2W:2 * R2W])
            nc.gpsimd.dma_start(out=nsq[:, :], in_=nsq_d[:, :])
            nc.gpsimd.dma_start(out=eye[:, :], in_=eye_d[:, :])
            nc.vector.memset(ones2[:, :], 1.0)

            # Gathered blocks -> SBUF full operands (block b at cols b*STRIP)
            for b in range(NDEV):
                rs = slice(b * STRIP, (b + 1) * STRIP)
                cs = slice(b * STRIP, (b + 1) * STRIP)
                nc.sync.dma_start_transpose(out=xG[:, cs],
                                            in_=cc_out[rs, 0:D])
                nc.sync.dma_start_transpose(out=yG[:, cs],
                                            in_=cc_out[rs, D:2 * D])

            # body emitted OPTS["repeat"] times (>1 only for HW timing:
            # outputs are identical per repeat, slope gives body time)
            for c in range(NCHUNK * OPTS["repeat"]):
                c = c % NCHUNK
                cs = slice(c * 128, (c + 1) * 128)
                for s in range(NSUP):
                    slot = s * NCHUNK + c       # acc layout: s-major
                    psK = pspool.tile([128, SUPER], f32, tag="ps")
                    psL = pspool.tile([128, SUPER], f32, tag="ps")
                    for t in range(NSUP):
                        jsl = slice(s * SUPER + t * TS, s * SUPER + (t + 1) * TS)
                        tsl = slice(t * TS, (t + 1) * TS)
                        nc.tensor.matmul(psK[:, tsl], lhsT=xTs[:, cs],
                                         rhs=xG[:, jsl], start=True, stop=False)
                    for t in range(NSUP):
                        jsl = slice(s * SUPER + t * TS, s * SUPER + (t + 1) * TS)
                        tsl = slice(t * TS, (t + 1) * TS)
                        nc.tensor.matmul(psK[:, tsl], lhsT=ones2[:, :],
                                         rhs=r2x[:, jsl], start=False, stop=True)
                    K_sb = klpool.tile([128, SUPER], bf16, tag="K")
                    nc.scalar.activation(K_sb[:, :], psK[:, :], Exp,
                                         bias=nsq[:, c:c + 1], scale=2.0,
                                         accum_out=accK[:, slot:slot + 1])

                    for t in range(NSUP):
                        jsl = slice(s * SUPER + t * TS, s * SUPER + (t + 1) * TS)
                        tsl = slice(t * TS, (t + 1) * TS)
                        nc.tensor.matmul(psL[:, tsl], lhsT=yTs[:, cs],
                                         rhs=yG[:, jsl], start=True, stop=False)
                    for t in range(NSUP):
                        jsl = slice(s * SUPER + t * TS, s * SUPER + (t + 1) * TS)
                        tsl = slice(t * TS, (t + 1) * TS)
                        nc.tensor.matmul(psL[:, tsl], lhsT=ones2[:, :],
                                         rhs=r2y[:, jsl], start=False, stop=True)
                    L_sb = klpool.tile([128, SUPER], bf16, tag="L")
                    nc.scalar.activation(L_sb[:, :], psL[:, :], Exp,
                                         bias=nsq[:, NCHUNK + c:NCHUNK + c + 1],
                                         scale=2.0,
                                         accum_out=accL[:, slot:slot + 1])

                    scr = scrpool.tile([128, SUPER], bf16, tag="scr")
                    nc.vector.scalar_tensor_tensor(
                        out=scr[:, :], in0=K_sb[:, :], scalar=1.0,
                        in1=L_sb[:, :], op0=mult, op1=mult,
                        accum_out=accS[:, slot:slot + 1])

            # --- pass B: recompute diagonal blocks bit-identically from the
            # local strip and extract their diagonals ---
            psDK = pspool.tile([128, SUPER], f32, tag="ps")
            psDL = pspool.tile([128, SUPER], f32, tag="ps")
            for c in range(NCHUNK):
                cs = slice(c * 128, (c + 1) * 128)
                nc.tensor.matmul(psDK[:, cs], lhsT=xTs[:, cs], rhs=xTs[:, cs],
                                 start=True, stop=False)
                nc.tensor.matmul(psDK[:, cs], lhsT=ones2[:, :],
                                 rhs=r2x[:, M + c * 128:M + (c + 1) * 128],
                                 start=False, stop=True)
                nc.tensor.matmul(psDL[:, cs], lhsT=yTs[:, cs], rhs=yTs[:, cs],
                                 start=True, stop=False)
                nc.tensor.matmul(psDL[:, cs], lhsT=ones2[:, :],
                                 rhs=r2y[:, M + c * 128:M + (c + 1) * 128],
                                 start=False, stop=True)
            KD = klpool.tile([128, SUPER], bf16, tag="K")
            LD = klpool.tile([128, SUPER], bf16, tag="L")
            for c in range(NCHUNK):
                cs = slice(c * 128, (c + 1) * 128)
                nc.scalar.activation(KD[:, cs], psDK[:, cs], Exp,
                                     bias=nsq[:, c:c + 1], scale=2.0)
                nc.scalar.activation(LD[:, cs], psDL[:, cs], Exp,
                                     bias=nsq[:, NCHUNK + c:NCHUNK + c + 1],
                                     scale=2.0)
            scrD = scrpool.tile([128, SUPER], bf16, tag="scr")
            for c in range(NCHUNK):
                cs = slice(c * 128, (c + 1) * 128)
                nc.vector.scalar_tensor_tensor(
                    out=scrD[:, cs], in0=KD[:, cs], scalar=1.0,
                    in1=eye[:, :], op0=mult, op1=mult,
                    accum_out=diagK[:, c:c + 1])
                nc.vector.scalar_tensor_tensor(
                    out=scrD[:, cs], in0=LD[:, cs], scalar=1.0,
                    in1=eye[:, :], op0=mult, op1=mult,
                    accum_out=diagL[:, c:c + 1])

            # --- final reductions: out[:, c] = sum_s acc[:, s*8+c] - diag ---
            nc.vector.tensor_add(t1[:, :], accK[:, 0:8], accK[:, 8:16])
            nc.vector.tensor_add(t2[:, :], accK[:, 16:24], accK[:, 24:32])
            nc.vector.tensor_add(t1[:, :], t1[:, :], t2[:, :])
            nc.vector.tensor_sub(out_sb[:, 0:8], t1[:, :], diagK[:, :])

            nc.vector.tensor_add(u1[:, :], accL[:, 0:8], accL[:, 8:16])
            nc.vector.tensor_add(u2[:, :], accL[:, 16:24], accL[:, 24:32])
            nc.vector.tensor_add(u1[:, :], u1[:, :], u2[:, :])
            nc.vector.tensor_sub(out_sb[:, 8:16], u1[:, :], diagL[:, :])

            nc.vector.tensor_add(t1[:, :], accS[:, 0:8], accS[:, 8:16])
            nc.vector.tensor_add(t2[:, :], accS[:, 16:24], accS[:, 24:32])
            nc.vector.tensor_add(t1[:, :], t1[:, :], t2[:, :])
            nc.vector.tensor_mul(t2[:, :], diagK[:, :], diagL[:, :])
            nc.vector.tensor_sub(t1[:, :], t1[:, :], t2[:, :])
            nc.vector.tensor_reduce(out_sb[:, 16:17], t1[:, :],
                                    axis=mybir.AxisListType.X, op=add)

            nc.gpsimd.dma_start(out=out_d[:, :], in_=out_sb[:, :])

    nc.compile()
    return nc


def _get_program():
    key = tuple(sorted(OPTS.items()))
    if key not in _cache:
        _cache[key] = _build_program()
    return _cache[key]


_EYE = None


def _eye_input():
    global _EYE
    if _EYE is None:
        _EYE = np.tile(np.eye(128, dtype=BF16), (NDEV, 1))
    return _EYE


def prepare_inputs(x, y):
    """Build the concatenated (core-major axis 0) input arrays."""
    xb = np.asarray(x, dtype=np.float32).astype(BF16)
    yb = np.asarray(y, dtype=np.float32).astype(BF16)

    XYS = np.empty((NDEV * STRIP, 2 * D), dtype=BF16)
    XYS[:, 0:D] = xb
    XYS[:, D:2 * D] = yb
    R2 = np.empty((NDEV * 2, 2 * R2W), dtype=BF16)
    NSQ = np.empty((NDEV * 128, 2 * NCHUNK), dtype=np.float32)

    for off, ab in ((0, xb), (1, yb)):
        af = ab.astype(np.float32)
        sq = (af * af).sum(axis=1, dtype=np.float64)      # [M] f64
        v = -sq / 2.0
        hi = v.astype(BF16)
        lo = (v - hi.astype(np.float64)).astype(BF16)
        hilo = np.stack([hi, lo], axis=0)                 # [2, M] bf16
        nsqf = (-sq).astype(np.float32)                   # [M] f32
        for dev in range(NDEV):
            sl = slice(dev * STRIP, (dev + 1) * STRIP)
            r2block = R2[dev * 2:(dev + 1) * 2]
            r2block[:, off * R2W:off * R2W + M] = hilo
            r2block[:, off * R2W + M:(off + 1) * R2W] = hilo[:, sl]
            NSQ[dev * 128:(dev + 1) * 128,
                off * NCHUNK:(off + 1) * NCHUNK] = \
                nsqf[sl].reshape(NCHUNK, 128).T
    return {"xys": XYS, "r2": R2, "nsq": NSQ}


def combine(out_all):
    """Host-side unshard + closed-form diagonal. float64 combine.

    out_all: [NDEV, 128, 17] f32 device results.
    """
    out_all = np.asarray(out_all, dtype=np.float64)
    rK = np.ones(M, dtype=np.float64)
    rL = np.ones(M, dtype=np.float64)
    for dev in range(NDEV):
        sl = slice(dev * STRIP, (dev + 1) * STRIP)
        rK[sl] += out_all[dev, :, 0:8].T.reshape(STRIP)
        rL[sl] += out_all[dev, :, 8:16].T.reshape(STRIP)
    S_lk = float(M) + out_all[:, :, 16].sum()
    S_K = rK.sum()
    S_L = rL.sum()
    dotRR = (rK * rL).sum()
    hsic = (S_lk - 2.0 * dotRR / M + S_K * S_L / (float(M) ** 2)) \
        / float((M - 1) ** 2)
    return np.float32(hsic)


def _get_runner():
    """Build (once) a cached jitted SPMD runner over the 8 cores.

    Constant inputs (eye) and the dummy output operand buffers are
    device-resident and reused across calls; per-call work is only the
    3 data-dependent input transfers, dispatch, and one small fetch.
    """
    rkey = ("runner",) + tuple(sorted(OPTS.items()))
    if rkey in _cache:
        return _cache[rkey]
    import jax
    import numpy as _np
    from jax.sharding import Mesh, PartitionSpec, NamedSharding
    from jax.experimental.shard_map import shard_map
    from concourse import bass2jax as b2j
    import concourse.mybir as mybir

    b2j.install_neuronx_cc_hook()
    nc = _get_program()

    partition_name = (nc.partition_id_tensor.name
                      if nc.partition_id_tensor else None)
    in_names, out_names, out_avals, zero_outs = [], [], [], []
    for alloc in nc.m.functions[0].allocations:
        if not isinstance(alloc, mybir.MemoryLocationSet):
            continue
        name = alloc.memorylocations[0].name
        if alloc.kind == "ExternalInput":
            if name != partition_name:
                in_names.append(name)
        elif alloc.kind == "ExternalOutput":
            out_names.append(name)
            np_dt = mybir.dt.np(alloc.dtype)
            out_avals.append(jax.core.ShapedArray(
                tuple(alloc.tensor_shape), np_dt))
            zero_outs.append(_np.zeros(tuple(alloc.tensor_shape), np_dt))

    n_params = len(in_names)
    all_names = list(in_names) + list(out_names)
    if partition_name is not None:
        all_names = all_names + [partition_name]

    def _body(*args):
        operands = list(args)
        if partition_name is not None:
            operands.append(b2j.partition_id_tensor())
        outs = b2j._bass_exec_p.bind(
            *operands,
            out_avals=tuple(out_avals),
            in_names=tuple(all_names),
            out_names=tuple(out_names),
            lowering_input_output_aliases=(),
            sim_require_finite=True,
            sim_require_nnan=True,
            nc=nc,
        )
        return tuple(outs)

    devices = jax.devices()[:NDEV]
    mesh = Mesh(_np.asarray(devices), ("core",))
    sharding = NamedSharding(mesh, PartitionSpec("core"))
    n_ops = n_params + len(out_names)
    sharded = jax.jit(
        shard_map(_body, mesh=mesh,
                  in_specs=(PartitionSpec("core"),) * n_ops,
                  out_specs=(PartitionSpec("core"),) * len(out_names),
                  check_rep=False),
        keep_unused=True)

    # Device-resident constants: dummy output operands + the eye input.
    zero_dev = [
        jax.device_put(_np.zeros((NDEV * z.shape[0], *z.shape[1:]), z.dtype),
                       sharding)
        for z in zero_outs
    ]
    const_dev = {"eye": jax.device_put(_eye_input(), sharding)}

    _cache[rkey] = (sharded, in_names, out_names, out_avals, zero_dev,
                    const_dev, sharding)
    return _cache[rkey]


def run_device(arrays):
    """Run the SPMD program; returns out array [NDEV, 128, 17]."""
    import jax
    (sharded, in_names, out_names, out_avals, zero_dev, const_dev,
     sharding) = _get_runner()
    dev_in = [const_dev[nm] if nm in const_dev
              else jax.device_put(arrays[nm], sharding)
              for nm in in_names]
    out_arrs = sharded(*dev_in, *zero_dev)
    out = np.asarray(out_arrs[0])
    return out.reshape(NDEV, *out_avals[0].shape)


def kernel(x, y):
    arrays = prepare_inputs(x, y)
    out = run_device(arrays)
    return combine(out)


def _timed_run(arrays, iters):
    """Min wall seconds for one dispatch of the current OPTS program."""
    import jax
    import time as _time
    (sharded, in_names, out_names, out_avals, zero_dev, const_dev,
     sharding) = _get_runner()
    dev_in = [const_dev[nm] if nm in const_dev
              else jax.device_put(arrays[nm], sharding)
              for nm in in_names]
    jax.block_until_ready(dev_in)
    best = float("inf")
    for i in range(iters + 1):
        t0 = _time.perf_counter()
        outs = sharded(*dev_in, *zero_dev)
        [np.asarray(o) for o in outs]
        dt = _time.perf_counter() - t0
        if i > 0:  # skip warm-up/compile call
            best = min(best, dt)
    return best


def time_on_hw(arrays, r_small=1, r_big=17, iters=8):
    """Estimate per-body HW time: (wall[R=r_big] - wall[R=r_small]) /
    (r_big - r_small), where R is the in-program body repeat count."""
    saved = OPTS["repeat"]
    walls = {}
    try:
        for r in (r_small, r_big):
            OPTS["repeat"] = r
            walls[r] = _timed_run(arrays, iters)
    finally:
        OPTS["repeat"] = saved
    per_body = (walls[r_big] - walls[r_small]) / (r_big - r_small)
    return per_body * 1e9, walls


# revision 13
# speedup vs baseline: 1.0372x; 1.0372x over previous
"""HSIC loss kernel for Trainium2, SPMD over 8 NeuronCores.

Math (reference): K = exp(-d2(x)), L = exp(-d2(y)),
  hsic = (sum(L*K) - 2*dot(rK,rL)/m + sum(K)*sum(L)/m^2) / (m-1)^2
where rK_i = sum_j K_ij (row sums; K, L symmetric).

Sharding: rows of the Gram matrices are split into 8 strips of 1024.
Each core receives ONLY its own strip of x and y (transposed, bf16)
plus tiny per-row metadata; the full x^T/y^T moving operands are
assembled on-device with an AllGather collective. This keeps the
host->device wire traffic at ~5 MB/call (vs ~39 MB if every core's
full rotated copy were shipped from the host), which dominates the
end-to-end latency on the axon-tunneled PJRT transport.

Per core, the [1024, 8192] strips of K and L are computed fully fused
(never materialized in DRAM):
  PSUM = x_strip @ x_full^T  (bf16 matmul, D=128 contraction)
         + rank-2 correction folding in -sq_j/2 (bf16 hi/lo split)
  K    = ACT exp(2*PSUM - sq_i)  (per-partition bias, scale=2)
The diagonal needs exact treatment (off-diagonal entries are ~e-30;
the diagonal K_ii = 1 carries the whole answer). Because the strips
are gathered in natural order, the diagonal block position would be
core-dependent, which a static SPMD program cannot address. Instead
the main pass INCLUDES the (slightly inexact) diagonal, and a second
tiny pass recomputes the 8 diagonal [128,128] blocks bit-identically
from the local strip (same operand values, same accumulation order),
extracts their diagonals, and subtracts them from the row sums and
the K*L sum. The true diagonal (exp(0)=1) is re-added analytically
on the host - exact math, not an approximation.

Per-core output is a single [128, 17] f32 tensor: row sums of K and
L by chunk (diag excluded) and the K*L partial sum. Host combines in
float64.
"""

import numpy as np
import ml_dtypes

BF16 = ml_dtypes.bfloat16

M = 8192
D = 128
NDEV = 8
STRIP = M // NDEV          # 1024 rows per core
NCHUNK = STRIP // 128      # 8 partition chunks per strip
SUPER = 2048               # ACT/PSUM super-tile width (4 PSUM banks)
NSUP = M // SUPER          # 4 j-supers
TS = 512                   # matmul free-dim tile (one PSUM bank)

R2W = M + STRIP            # 9216: full-M correction row + own-strip slice
NSLOT = NCHUNK * NSUP      # 32 accumulation slots

_cache = {}

OPTS = {"repeat": 1}


def _build_program():
    import concourse.bacc as bacc
    import concourse.mybir as mybir
    from concourse import tile

    f32 = mybir.dt.float32
    bf16 = mybir.dt.bfloat16
    Exp = mybir.ActivationFunctionType.Exp
    mult = mybir.AluOpType.mult
    add = mybir.AluOpType.add

    nc = bacc.Bacc("TRN2", target_bir_lowering=False, debug=False,
                   num_devices=NDEV)

    # DRAM inputs (per-core values differ, same shapes: SPMD)
    # meta rows (x256 bf16): [0:36] r2x hi, [36:72] r2x lo, [72:108] r2y hi,
    # [108:144] r2y lo, [144:152] nsq hi, [152:160] nsq lo.
    xys_d = nc.dram_tensor("xys", [STRIP, 2 * D], bf16, kind="ExternalInput")
    meta_d = nc.dram_tensor("meta", [160, 256], bf16, kind="ExternalInput")
    eye_d = nc.dram_tensor("eye", [128, 128], bf16, kind="ExternalInput")

    out_d = nc.dram_tensor("out", [128, 17], f32, kind="ExternalOutput")

    with tile.TileContext(nc) as tc:
        with (
            tc.tile_pool(name="dram", bufs=1, space="DRAM") as dram,
            tc.tile_pool(name="const", bufs=1) as cpool,
            tc.tile_pool(name="psum", bufs=2, space="PSUM") as pspool,
            tc.tile_pool(name="kl", bufs=2) as klpool,
            tc.tile_pool(name="scr", bufs=2) as scrpool,
        ):
            # --- AllGather the x/y strips into full moving operands ---
            cc_in = dram.tile([STRIP, 2 * D], bf16)
            cc_out = dram.tile([NDEV * STRIP, 2 * D], bf16)
            nc.gpsimd.dma_start(out=cc_in[:, :], in_=xys_d[:, :])
            nc.gpsimd.collective_compute(
                "AllGather",
                mybir.AluOpType.bypass,
                replica_groups=[list(range(NDEV))],
                ins=[cc_in.opt()],
                outs=[cc_out.opt()],
            )

            xTs = cpool.tile([128, STRIP], bf16, tag="xTs")
            yTs = cpool.tile([128, STRIP], bf16, tag="yTs")
            r2x = cpool.tile([2, R2W], bf16, tag="r2x")
            r2y = cpool.tile([2, R2W], bf16, tag="r2y")
            nsq = cpool.tile([128, 2 * NCHUNK], f32, tag="nsq")
            nsqh = cpool.tile([128, 2 * NCHUNK], bf16, tag="nsqh")
            nsql = cpool.tile([128, 2 * NCHUNK], bf16, tag="nsql")
            nsqhf = cpool.tile([128, 2 * NCHUNK], f32, tag="nsqhf")
            nsqlf = cpool.tile([128, 2 * NCHUNK], f32, tag="nsqlf")
            eye = cpool.tile([128, 128], bf16, tag="eye")
            ones2 = cpool.tile([2, D], bf16, tag="ones2")
            xG = cpool.tile([128, M], bf16, tag="xG")
            yG = cpool.tile([128, M], bf16, tag="yG")
            accK = cpool.tile([128, NSLOT], f32, tag="accK")
            accL = cpool.tile([128, NSLOT], f32, tag="accL")
            accS = cpool.tile([128, NSLOT], f32, tag="accS")
            diagK = cpool.tile([128, NCHUNK], f32, tag="diagK")
            diagL = cpool.tile([128, NCHUNK], f32, tag="diagL")
            out_sb = cpool.tile([128, 17], f32, tag="out")
            t1 = cpool.tile([128, NCHUNK], f32, tag="t1")
            t2 = cpool.tile([128, NCHUNK], f32, tag="t2")
            u1 = cpool.tile([128, NCHUNK], f32, tag="u1")
            u2 = cpool.tile([128, NCHUNK], f32, tag="u2")

            nc.sync.dma_start_transpose(out=xTs[:, :], in_=xys_d[:, 0:D])
            nc.sync.dma_start_transpose(out=yTs[:, :], in_=xys_d[:, D:2 * D])
            nc.gpsimd.dma_start(out=r2x[0:1, :], in_=meta_d[0:36, :])
            nc.gpsimd.dma_start(out=r2x[1:2, :], in_=meta_d[36:72, :])
            nc.gpsimd.dma_start(out=r2y[0:1, :], in_=meta_d[72:108, :])
            nc.gpsimd.dma_start(out=r2y[1:2, :], in_=meta_d[108:144, :])
            nc.gpsimd.dma_start(out=nsqh[:, :], in_=meta_d[144:152, :])
            nc.gpsimd.dma_start(out=nsql[:, :], in_=meta_d[152:160, :])
            nc.gpsimd.dma_start(out=eye[:, :], in_=eye_d[:, :])
            nc.vector.memset(ones2[:, :], 1.0)
            nc.vector.tensor_copy(nsqhf[:, :], nsqh[:, :])
            nc.vector.tensor_copy(nsqlf[:, :], nsql[:, :])
            nc.vector.tensor_add(nsq[:, :], nsqhf[:, :], nsqlf[:, :])

            # Gathered blocks -> SBUF full operands (block b at cols b*STRIP)
            for b in range(NDEV):
                rs = slice(b * STRIP, (b + 1) * STRIP)
                cs = slice(b * STRIP, (b + 1) * STRIP)
                nc.sync.dma_start_transpose(out=xG[:, cs],
                                            in_=cc_out[rs, 0:D])
                nc.sync.dma_start_transpose(out=yG[:, cs],
                                            in_=cc_out[rs, D:2 * D])

            # body emitted OPTS["repeat"] times (>1 only for HW timing:
            # outputs are identical per repeat, slope gives body time)
            for c in range(NCHUNK * OPTS["repeat"]):
                c = c % NCHUNK
                cs = slice(c * 128, (c + 1) * 128)
                for s in range(NSUP):
                    slot = s * NCHUNK + c       # acc layout: s-major
                    psK = pspool.tile([128, SUPER], f32, tag="ps")
                    psL = pspool.tile([128, SUPER], f32, tag="ps")
                    for t in range(NSUP):
                        jsl = slice(s * SUPER + t * TS, s * SUPER + (t + 1) * TS)
                        tsl = slice(t * TS, (t + 1) * TS)
                        nc.tensor.matmul(psK[:, tsl], lhsT=xTs[:, cs],
                                         rhs=xG[:, jsl], start=True, stop=False)
                    for t in range(NSUP):
                        jsl = slice(s * SUPER + t * TS, s * SUPER + (t + 1) * TS)
                        tsl = slice(t * TS, (t + 1) * TS)
                        nc.tensor.matmul(psK[:, tsl], lhsT=ones2[:, :],
                                         rhs=r2x[:, jsl], start=False, stop=True)
                    K_sb = klpool.tile([128, SUPER], bf16, tag="K")
                    nc.scalar.activation(K_sb[:, :], psK[:, :], Exp,
                                         bias=nsq[:, c:c + 1], scale=2.0,
                                         accum_out=accK[:, slot:slot + 1])

                    for t in range(NSUP):
                        jsl = slice(s * SUPER + t * TS, s * SUPER + (t + 1) * TS)
                        tsl = slice(t * TS, (t + 1) * TS)
                        nc.tensor.matmul(psL[:, tsl], lhsT=yTs[:, cs],
                                         rhs=yG[:, jsl], start=True, stop=False)
                    for t in range(NSUP):
                        jsl = slice(s * SUPER + t * TS, s * SUPER + (t + 1) * TS)
                        tsl = slice(t * TS, (t + 1) * TS)
                        nc.tensor.matmul(psL[:, tsl], lhsT=ones2[:, :],
                                         rhs=r2y[:, jsl], start=False, stop=True)
                    L_sb = klpool.tile([128, SUPER], bf16, tag="L")
                    nc.scalar.activation(L_sb[:, :], psL[:, :], Exp,
                                         bias=nsq[:, NCHUNK + c:NCHUNK + c + 1],
                                         scale=2.0,
                                         accum_out=accL[:, slot:slot + 1])

                    scr = scrpool.tile([128, SUPER], bf16, tag="scr")
                    nc.vector.scalar_tensor_tensor(
                        out=scr[:, :], in0=K_sb[:, :], scalar=1.0,
                        in1=L_sb[:, :], op0=mult, op1=mult,
                        accum_out=accS[:, slot:slot + 1])

            # --- pass B: recompute diagonal blocks bit-identically from the
            # local strip and extract their diagonals ---
            psDK = pspool.tile([128, SUPER], f32, tag="ps")
            psDL = pspool.tile([128, SUPER], f32, tag="ps")
            for c in range(NCHUNK):
                cs = slice(c * 128, (c + 1) * 128)
                nc.tensor.matmul(psDK[:, cs], lhsT=xTs[:, cs], rhs=xTs[:, cs],
                                 start=True, stop=False)
                nc.tensor.matmul(psDK[:, cs], lhsT=ones2[:, :],
                                 rhs=r2x[:, M + c * 128:M + (c + 1) * 128],
                                 start=False, stop=True)
                nc.tensor.matmul(psDL[:, cs], lhsT=yTs[:, cs], rhs=yTs[:, cs],
                                 start=True, stop=False)
                nc.tensor.matmul(psDL[:, cs], lhsT=ones2[:, :],
                                 rhs=r2y[:, M + c * 128:M + (c + 1) * 128],
                                 start=False, stop=True)
            KD = klpool.tile([128, SUPER], bf16, tag="K")
            LD = klpool.tile([128, SUPER], bf16, tag="L")
            for c in range(NCHUNK):
                cs = slice(c * 128, (c + 1) * 128)
                nc.scalar.activation(KD[:, cs], psDK[:, cs], Exp,
                                     bias=nsq[:, c:c + 1], scale=2.0)
                nc.scalar.activation(LD[:, cs], psDL[:, cs], Exp,
                                     bias=nsq[:, NCHUNK + c:NCHUNK + c + 1],
                                     scale=2.0)
            scrD = scrpool.tile([128, SUPER], bf16, tag="scr")
            for c in range(NCHUNK):
                cs = slice(c * 128, (c + 1) * 128)
                nc.vector.scalar_tensor_tensor(
                    out=scrD[:, cs], in0=KD[:, cs], scalar=1.0,
                    in1=eye[:, :], op0=mult, op1=mult,
                    accum_out=diagK[:, c:c + 1])
                nc.vector.scalar_tensor_tensor(
                    out=scrD[:, cs], in0=LD[:, cs], scalar=1.0,
                    in1=eye[:, :], op0=mult, op1=mult,
                    accum_out=diagL[:, c:c + 1])

            # --- final reductions: out[:, c] = sum_s acc[:, s*8+c] - diag ---
            nc.vector.tensor_add(t1[:, :], accK[:, 0:8], accK[:, 8:16])
            nc.vector.tensor_add(t2[:, :], accK[:, 16:24], accK[:, 24:32])
            nc.vector.tensor_add(t1[:, :], t1[:, :], t2[:, :])
            nc.vector.tensor_sub(out_sb[:, 0:8], t1[:, :], diagK[:, :])

            nc.vector.tensor_add(u1[:, :], accL[:, 0:8], accL[:, 8:16])
            nc.vector.tensor_add(u2[:, :], accL[:, 16:24], accL[:, 24:32])
            nc.vector.tensor_add(u1[:, :], u1[:, :], u2[:, :])
            nc.vector.tensor_sub(out_sb[:, 8:16], u1[:, :], diagL[:, :])

            nc.vector.tensor_add(t1[:, :], accS[:, 0:8], accS[:, 8:16])
            nc.vector.tensor_add(t2[:, :], accS[:, 16:24], accS[:, 24:32])
            nc.vector.tensor_add(t1[:, :], t1[:, :], t2[:, :])
            nc.vector.tensor_mul(t2[:, :], diagK[:, :], diagL[:, :])
            nc.vector.tensor_sub(t1[:, :], t1[:, :], t2[:, :])
            nc.vector.tensor_reduce(out_sb[:, 16:17], t1[:, :],
                                    axis=mybir.AxisListType.X, op=add)

            nc.gpsimd.dma_start(out=out_d[:, :], in_=out_sb[:, :])

    nc.compile()
    return nc


def _get_program():
    key = tuple(sorted(OPTS.items()))
    if key not in _cache:
        _cache[key] = _build_program()
    return _cache[key]


_EYE = None


def _eye_input():
    global _EYE
    if _EYE is None:
        _EYE = np.tile(np.eye(128, dtype=BF16), (NDEV, 1))
    return _EYE


def _f32_to_hilo(v64):
    hi = v64.astype(BF16)
    lo = (v64 - hi.astype(np.float64)).astype(BF16)
    return hi, lo


def prepare_xys(xb, yb):
    XYS = np.empty((NDEV * STRIP, 2 * D), dtype=BF16)
    XYS[:, 0:D] = xb
    XYS[:, D:2 * D] = yb
    return XYS


def prepare_meta(xb, yb):
    """Packed per-core metadata [NDEV*160, 256] bf16; see meta_d layout.

    meta rows per core: [0:36] r2x hi, [36:72] r2x lo, [72:108] r2y hi,
    [108:144] r2y lo, [144:152] nsq hi, [152:160] nsq lo. The nsq flat
    order matches the SBUF [128, 2*NCHUNK] tile iteration
    (partition-major): element (p, off*NCHUNK + c) = -sq[strip][c*128+p].
    """
    META = np.empty((NDEV, 160, 256), dtype=BF16)
    for off, ab in ((0, xb), (1, yb)):
        af = ab.astype(np.float32)
        sq = (af * af).sum(axis=1, dtype=np.float64)      # [M] f64
        hi, lo = _f32_to_hilo(-sq / 2.0)                  # [M] bf16 each
        nhi, nlo = _f32_to_hilo(-sq)                      # [M] bf16 each
        r2o = off * 72
        for dev in range(NDEV):
            sl = slice(dev * STRIP, (dev + 1) * STRIP)
            blk = META[dev]
            for rows, vec in ((blk[r2o:r2o + 36], hi),
                              (blk[r2o + 36:r2o + 72], lo)):
                flat = rows.reshape(R2W)
                flat[0:M] = vec
                flat[M:R2W] = vec[sl]
            for rows, vec in ((blk[144:152], nhi), (blk[152:160], nlo)):
                rows.reshape(128, 2 * NCHUNK)[
                    :, off * NCHUNK:(off + 1) * NCHUNK] = \
                    vec[sl].reshape(NCHUNK, 128).T
    return META.reshape(NDEV * 160, 256)


def prepare_inputs(x, y):
    xb = np.asarray(x, dtype=np.float32).astype(BF16)
    yb = np.asarray(y, dtype=np.float32).astype(BF16)
    return {"xys": prepare_xys(xb, yb), "meta": prepare_meta(xb, yb)}


def combine(out_all):
    """Host-side unshard + closed-form diagonal. float64 combine.

    out_all: [NDEV, 128, 17] f32 device results.
    """
    out_all = np.asarray(out_all, dtype=np.float64)
    rK = np.ones(M, dtype=np.float64)
    rL = np.ones(M, dtype=np.float64)
    for dev in range(NDEV):
        sl = slice(dev * STRIP, (dev + 1) * STRIP)
        rK[sl] += out_all[dev, :, 0:8].T.reshape(STRIP)
        rL[sl] += out_all[dev, :, 8:16].T.reshape(STRIP)
    S_lk = float(M) + out_all[:, :, 16].sum()
    S_K = rK.sum()
    S_L = rL.sum()
    dotRR = (rK * rL).sum()
    hsic = (S_lk - 2.0 * dotRR / M + S_K * S_L / (float(M) ** 2)) \
        / float((M - 1) ** 2)
    return np.float32(hsic)


def _get_runner():
    """Build (once) a cached jitted SPMD runner over the 8 cores.

    Constant inputs (eye) and the dummy output operand buffers are
    device-resident and reused across calls; per-call work is only the
    3 data-dependent input transfers, dispatch, and one small fetch.
    """
    rkey = ("runner",) + tuple(sorted(OPTS.items()))
    if rkey in _cache:
        return _cache[rkey]
    import jax
    import numpy as _np
    from jax.sharding import Mesh, PartitionSpec, NamedSharding
    from jax.experimental.shard_map import shard_map
    from concourse import bass2jax as b2j
    import concourse.mybir as mybir

    b2j.install_neuronx_cc_hook()
    nc = _get_program()

    partition_name = (nc.partition_id_tensor.name
                      if nc.partition_id_tensor else None)
    in_names, out_names, out_avals, zero_outs = [], [], [], []
    for alloc in nc.m.functions[0].allocations:
        if not isinstance(alloc, mybir.MemoryLocationSet):
            continue
        name = alloc.memorylocations[0].name
        if alloc.kind == "ExternalInput":
            if name != partition_name:
                in_names.append(name)
        elif alloc.kind == "ExternalOutput":
            out_names.append(name)
            np_dt = mybir.dt.np(alloc.dtype)
            out_avals.append(jax.core.ShapedArray(
                tuple(alloc.tensor_shape), np_dt))
            zero_outs.append(_np.zeros(tuple(alloc.tensor_shape), np_dt))

    n_params = len(in_names)
    all_names = list(in_names) + list(out_names)
    if partition_name is not None:
        all_names = all_names + [partition_name]

    def _body(*args):
        operands = list(args)
        if partition_name is not None:
            operands.append(b2j.partition_id_tensor())
        outs = b2j._bass_exec_p.bind(
            *operands,
            out_avals=tuple(out_avals),
            in_names=tuple(all_names),
            out_names=tuple(out_names),
            lowering_input_output_aliases=(),
            sim_require_finite=True,
            sim_require_nnan=True,
            nc=nc,
        )
        return tuple(outs)

    devices = jax.devices()[:NDEV]
    mesh = Mesh(_np.asarray(devices), ("core",))
    sharding = NamedSharding(mesh, PartitionSpec("core"))
    n_ops = n_params + len(out_names)
    sharded = jax.jit(
        shard_map(_body, mesh=mesh,
                  in_specs=(PartitionSpec("core"),) * n_ops,
                  out_specs=(PartitionSpec("core"),) * len(out_names),
                  check_rep=False),
        keep_unused=True)

    # Device-resident constants: dummy output operands + the eye input.
    zero_dev = [
        jax.device_put(_np.zeros((NDEV * z.shape[0], *z.shape[1:]), z.dtype),
                       sharding)
        for z in zero_outs
    ]
    const_dev = {"eye": jax.device_put(_eye_input(), sharding)}

    _cache[rkey] = (sharded, in_names, out_names, out_avals, zero_dev,
                    const_dev, sharding)
    return _cache[rkey]


def run_device(arrays):
    """Run the SPMD program; returns out array [NDEV, 128, 17]."""
    import jax
    (sharded, in_names, out_names, out_avals, zero_dev, const_dev,
     sharding) = _get_runner()
    dev_in = [const_dev[nm] if nm in const_dev
              else jax.device_put(arrays[nm], sharding)
              for nm in in_names]
    out_arrs = sharded(*dev_in, *zero_dev)
    out = np.asarray(out_arrs[0])
    return out.reshape(NDEV, *out_avals[0].shape)


def kernel(x, y):
    import jax
    (sharded, in_names, out_names, out_avals, zero_dev, const_dev,
     sharding) = _get_runner()
    xb = np.asarray(x, dtype=np.float32).astype(BF16)
    yb = np.asarray(y, dtype=np.float32).astype(BF16)
    # Enqueue the big transfer first so it streams while the host
    # computes the metadata array.
    staged = {"xys": jax.device_put(prepare_xys(xb, yb), sharding)}
    staged["meta"] = jax.device_put(prepare_meta(xb, yb), sharding)
    dev_in = [const_dev[nm] if nm in const_dev else staged[nm]
              for nm in in_names]
    out_arrs = sharded(*dev_in, *zero_dev)
    out = np.asarray(out_arrs[0]).reshape(NDEV, *out_avals[0].shape)
    return combine(out)


def _timed_run(arrays, iters):
    """Min wall seconds for one dispatch of the current OPTS program."""
    import jax
    import time as _time
    (sharded, in_names, out_names, out_avals, zero_dev, const_dev,
     sharding) = _get_runner()
    dev_in = [const_dev[nm] if nm in const_dev
              else jax.device_put(arrays[nm], sharding)
              for nm in in_names]
    jax.block_until_ready(dev_in)
    best = float("inf")
    for i in range(iters + 1):
        t0 = _time.perf_counter()
        outs = sharded(*dev_in, *zero_dev)
        [np.asarray(o) for o in outs]
        dt = _time.perf_counter() - t0
        if i > 0:  # skip warm-up/compile call
            best = min(best, dt)
    return best


def time_on_hw(arrays, r_small=1, r_big=17, iters=8):
    """Estimate per-body HW time: (wall[R=r_big] - wall[R=r_small]) /
    (r_big - r_small), where R is the in-program body repeat count."""
    saved = OPTS["repeat"]
    walls = {}
    try:
        for r in (r_small, r_big):
            OPTS["repeat"] = r
            walls[r] = _timed_run(arrays, iters)
    finally:
        OPTS["repeat"] = saved
    per_body = (walls[r_big] - walls[r_small]) / (r_big - r_small)
    return per_body * 1e9, walls


# revision 16
# speedup vs baseline: 1.2998x; 1.2532x over previous
"""HSIC loss kernel for Trainium2, SPMD over 8 NeuronCores.

Math (reference): K = exp(-d2(x)), L = exp(-d2(y)),
  hsic = (sum(L*K) - 2*dot(rK,rL)/m + sum(K)*sum(L)/m^2) / (m-1)^2
where rK_i = sum_j K_ij (row sums; K, L symmetric).

Sharding: rows of the Gram matrices are split into 8 strips of 1024.
Each core receives ONLY its own strip of x and y (transposed, bf16)
plus tiny per-row metadata; the full x^T/y^T moving operands are
assembled on-device with an AllGather collective. This keeps the
host->device wire traffic at ~5 MB/call (vs ~39 MB if every core's
full rotated copy were shipped from the host), which dominates the
end-to-end latency on the axon-tunneled PJRT transport.

Per core, the [1024, 8192] strips of K and L are computed fully fused
(never materialized in DRAM):
  PSUM = x_strip @ x_full^T  (bf16 matmul, D=128 contraction)
         + rank-2 correction folding in -sq_j/2 (bf16 hi/lo split)
  K    = ACT exp(2*PSUM - sq_i)  (per-partition bias, scale=2)
The diagonal needs exact treatment (off-diagonal entries are ~e-30;
the diagonal K_ii = 1 carries the whole answer). Because the strips
are gathered in natural order, the diagonal block position would be
core-dependent, which a static SPMD program cannot address. Instead
the main pass INCLUDES the (slightly inexact) diagonal, and a second
tiny pass recomputes the 8 diagonal [128,128] blocks bit-identically
from the local strip (same operand values, same accumulation order),
extracts their diagonals, and subtracts them from the row sums and
the K*L sum. The true diagonal (exp(0)=1) is re-added analytically
on the host - exact math, not an approximation.

Per-core output is a single [128, 17] f32 tensor: row sums of K and
L by chunk (diag excluded) and the K*L partial sum. Host combines in
float64.
"""

import numpy as np
import ml_dtypes

BF16 = ml_dtypes.bfloat16
FP8 = ml_dtypes.float8_e4m3

M = 8192
D = 128
NDEV = 8
STRIP = M // NDEV          # 1024 rows per core
NCHUNK = STRIP // 128      # 8 partition chunks per strip
SUPER = 2048               # ACT/PSUM super-tile width (4 PSUM banks)
NSUP = M // SUPER          # 4 j-supers
TS = 512                   # matmul free-dim tile (one PSUM bank)

R2W = M + STRIP            # 9216: full-M correction row + own-strip slice
NSLOT = NCHUNK * NSUP      # 32 accumulation slots

_cache = {}

OPTS = {"repeat": 1}


def _build_program():
    import concourse.bacc as bacc
    import concourse.mybir as mybir
    from concourse import tile

    f32 = mybir.dt.float32
    bf16 = mybir.dt.bfloat16
    f8 = mybir.dt.float8e4
    Exp = mybir.ActivationFunctionType.Exp
    mult = mybir.AluOpType.mult
    add = mybir.AluOpType.add

    nc = bacc.Bacc("TRN2", target_bir_lowering=False, debug=False,
                   num_devices=NDEV)

    # DRAM inputs (per-core values differ, same shapes: SPMD)
    # meta rows (x256 bf16): [0:36] r2x hi, [36:72] r2x lo, [72:108] r2y hi,
    # [108:144] r2y lo, [144:152] nsq hi, [152:160] nsq lo.
    xys_d = nc.dram_tensor("xys", [128, 2 * STRIP], f8, kind="ExternalInput")
    meta_d = nc.dram_tensor("meta", [160, 256], bf16, kind="ExternalInput")
    eye_d = nc.dram_tensor("eye", [128, 128], bf16, kind="ExternalInput")

    out_d = nc.dram_tensor("out", [128, 17], f32, kind="ExternalOutput")

    with tile.TileContext(nc) as tc:
        with (
            tc.tile_pool(name="dram", bufs=1, space="DRAM") as dram,
            tc.tile_pool(name="const", bufs=1) as cpool,
            tc.tile_pool(name="psum", bufs=2, space="PSUM") as pspool,
            tc.tile_pool(name="kl", bufs=2) as klpool,
            tc.tile_pool(name="scr", bufs=2) as scrpool,
        ):
            # --- AllGather the x/y strips into full moving operands ---
            cc_in = dram.tile([128, 2 * STRIP], f8)
            cc_out = dram.tile([NDEV * 128, 2 * STRIP], f8)
            nc.gpsimd.dma_start(out=cc_in[:, :], in_=xys_d[:, :])
            nc.gpsimd.collective_compute(
                "AllGather",
                mybir.AluOpType.bypass,
                replica_groups=[list(range(NDEV))],
                ins=[cc_in.opt()],
                outs=[cc_out.opt()],
            )

            xys = cpool.tile([128, 2 * STRIP], f8, tag="xys")
            r2x = cpool.tile([2, R2W], bf16, tag="r2x")
            r2y = cpool.tile([2, R2W], bf16, tag="r2y")
            nsq = cpool.tile([128, 2 * NCHUNK], f32, tag="nsq")
            nsqh = cpool.tile([128, 2 * NCHUNK], bf16, tag="nsqh")
            nsql = cpool.tile([128, 2 * NCHUNK], bf16, tag="nsql")
            nsqhf = cpool.tile([128, 2 * NCHUNK], f32, tag="nsqhf")
            nsqlf = cpool.tile([128, 2 * NCHUNK], f32, tag="nsqlf")
            eye = cpool.tile([128, 128], bf16, tag="eye")
            ones2 = cpool.tile([2, D], bf16, tag="ones2")
            xG = cpool.tile([128, M], f8, tag="xG")
            yG = cpool.tile([128, M], f8, tag="yG")
            accK = cpool.tile([128, NSLOT], f32, tag="accK")
            accL = cpool.tile([128, NSLOT], f32, tag="accL")
            accS = cpool.tile([128, NSLOT], f32, tag="accS")
            diagK = cpool.tile([128, NCHUNK], f32, tag="diagK")
            diagL = cpool.tile([128, NCHUNK], f32, tag="diagL")
            out_sb = cpool.tile([128, 17], f32, tag="out")
            t1 = cpool.tile([128, NCHUNK], f32, tag="t1")
            t2 = cpool.tile([128, NCHUNK], f32, tag="t2")
            u1 = cpool.tile([128, NCHUNK], f32, tag="u1")
            u2 = cpool.tile([128, NCHUNK], f32, tag="u2")

            nc.gpsimd.dma_start(out=xys[:, :], in_=xys_d[:, :])
            nc.gpsimd.dma_start(out=r2x[0:1, :], in_=meta_d[0:36, :])
            nc.gpsimd.dma_start(out=r2x[1:2, :], in_=meta_d[36:72, :])
            nc.gpsimd.dma_start(out=r2y[0:1, :], in_=meta_d[72:108, :])
            nc.gpsimd.dma_start(out=r2y[1:2, :], in_=meta_d[108:144, :])
            nc.gpsimd.dma_start(out=nsqh[:, :], in_=meta_d[144:152, :])
            nc.gpsimd.dma_start(out=nsql[:, :], in_=meta_d[152:160, :])
            nc.gpsimd.dma_start(out=eye[:, :], in_=eye_d[:, :])
            nc.vector.memset(ones2[:, :], 1.0)
            nc.vector.tensor_copy(nsqhf[:, :], nsqh[:, :])
            nc.vector.tensor_copy(nsqlf[:, :], nsql[:, :])
            nc.vector.tensor_add(nsq[:, :], nsqhf[:, :], nsqlf[:, :])

            # Gathered blocks -> SBUF full operands (block b at cols b*STRIP)
            for b in range(NDEV):
                rs = slice(b * 128, (b + 1) * 128)
                cs = slice(b * STRIP, (b + 1) * STRIP)
                nc.gpsimd.dma_start(out=xG[:, cs], in_=cc_out[rs, 0:STRIP])
                nc.gpsimd.dma_start(out=yG[:, cs],
                                    in_=cc_out[rs, STRIP:2 * STRIP])

            xTs = xys[:, 0:STRIP]
            yTs = xys[:, STRIP:2 * STRIP]

            # body emitted OPTS["repeat"] times (>1 only for HW timing:
            # outputs are identical per repeat, slope gives body time)
            for c in range(NCHUNK * OPTS["repeat"]):
                c = c % NCHUNK
                cs = slice(c * 128, (c + 1) * 128)
                for s in range(NSUP):
                    slot = s * NCHUNK + c       # acc layout: s-major
                    psK = pspool.tile([128, SUPER], f32, tag="ps")
                    psL = pspool.tile([128, SUPER], f32, tag="ps")
                    for t in range(NSUP):
                        jsl = slice(s * SUPER + t * TS, s * SUPER + (t + 1) * TS)
                        tsl = slice(t * TS, (t + 1) * TS)
                        nc.tensor.matmul(psK[:, tsl], lhsT=xTs[:, cs],
                                         rhs=xG[:, jsl], start=True, stop=False)
                    for t in range(NSUP):
                        jsl = slice(s * SUPER + t * TS, s * SUPER + (t + 1) * TS)
                        tsl = slice(t * TS, (t + 1) * TS)
                        nc.tensor.matmul(psK[:, tsl], lhsT=ones2[:, :],
                                         rhs=r2x[:, jsl], start=False, stop=True)
                    K_sb = klpool.tile([128, SUPER], bf16, tag="K")
                    nc.scalar.activation(K_sb[:, :], psK[:, :], Exp,
                                         bias=nsq[:, c:c + 1], scale=2.0,
                                         accum_out=accK[:, slot:slot + 1])

                    for t in range(NSUP):
                        jsl = slice(s * SUPER + t * TS, s * SUPER + (t + 1) * TS)
                        tsl = slice(t * TS, (t + 1) * TS)
                        nc.tensor.matmul(psL[:, tsl], lhsT=yTs[:, cs],
                                         rhs=yG[:, jsl], start=True, stop=False)
                    for t in range(NSUP):
                        jsl = slice(s * SUPER + t * TS, s * SUPER + (t + 1) * TS)
                        tsl = slice(t * TS, (t + 1) * TS)
                        nc.tensor.matmul(psL[:, tsl], lhsT=ones2[:, :],
                                         rhs=r2y[:, jsl], start=False, stop=True)
                    L_sb = klpool.tile([128, SUPER], bf16, tag="L")
                    nc.scalar.activation(L_sb[:, :], psL[:, :], Exp,
                                         bias=nsq[:, NCHUNK + c:NCHUNK + c + 1],
                                         scale=2.0,
                                         accum_out=accL[:, slot:slot + 1])

                    scr = scrpool.tile([128, SUPER], bf16, tag="scr")
                    nc.vector.scalar_tensor_tensor(
                        out=scr[:, :], in0=K_sb[:, :], scalar=1.0,
                        in1=L_sb[:, :], op0=mult, op1=mult,
                        accum_out=accS[:, slot:slot + 1])

            # --- pass B: recompute diagonal blocks bit-identically from the
            # local strip and extract their diagonals ---
            psDK = pspool.tile([128, SUPER], f32, tag="ps")
            psDL = pspool.tile([128, SUPER], f32, tag="ps")
            for c in range(NCHUNK):
                cs = slice(c * 128, (c + 1) * 128)
                nc.tensor.matmul(psDK[:, cs], lhsT=xTs[:, cs], rhs=xTs[:, cs],
                                 start=True, stop=False)
                nc.tensor.matmul(psDK[:, cs], lhsT=ones2[:, :],
                                 rhs=r2x[:, M + c * 128:M + (c + 1) * 128],
                                 start=False, stop=True)
                nc.tensor.matmul(psDL[:, cs], lhsT=yTs[:, cs], rhs=yTs[:, cs],
                                 start=True, stop=False)
                nc.tensor.matmul(psDL[:, cs], lhsT=ones2[:, :],
                                 rhs=r2y[:, M + c * 128:M + (c + 1) * 128],
                                 start=False, stop=True)
            KD = klpool.tile([128, SUPER], bf16, tag="K")
            LD = klpool.tile([128, SUPER], bf16, tag="L")
            for c in range(NCHUNK):
                cs = slice(c * 128, (c + 1) * 128)
                nc.scalar.activation(KD[:, cs], psDK[:, cs], Exp,
                                     bias=nsq[:, c:c + 1], scale=2.0)
                nc.scalar.activation(LD[:, cs], psDL[:, cs], Exp,
                                     bias=nsq[:, NCHUNK + c:NCHUNK + c + 1],
                                     scale=2.0)
            scrD = scrpool.tile([128, SUPER], bf16, tag="scr")
            for c in range(NCHUNK):
                cs = slice(c * 128, (c + 1) * 128)
                nc.vector.scalar_tensor_tensor(
                    out=scrD[:, cs], in0=KD[:, cs], scalar=1.0,
                    in1=eye[:, :], op0=mult, op1=mult,
                    accum_out=diagK[:, c:c + 1])
                nc.vector.scalar_tensor_tensor(
                    out=scrD[:, cs], in0=LD[:, cs], scalar=1.0,
                    in1=eye[:, :], op0=mult, op1=mult,
                    accum_out=diagL[:, c:c + 1])

            # --- final reductions: out[:, c] = sum_s acc[:, s*8+c] - diag ---
            nc.vector.tensor_add(t1[:, :], accK[:, 0:8], accK[:, 8:16])
            nc.vector.tensor_add(t2[:, :], accK[:, 16:24], accK[:, 24:32])
            nc.vector.tensor_add(t1[:, :], t1[:, :], t2[:, :])
            nc.vector.tensor_sub(out_sb[:, 0:8], t1[:, :], diagK[:, :])

            nc.vector.tensor_add(u1[:, :], accL[:, 0:8], accL[:, 8:16])
            nc.vector.tensor_add(u2[:, :], accL[:, 16:24], accL[:, 24:32])
            nc.vector.tensor_add(u1[:, :], u1[:, :], u2[:, :])
            nc.vector.tensor_sub(out_sb[:, 8:16], u1[:, :], diagL[:, :])

            nc.vector.tensor_add(t1[:, :], accS[:, 0:8], accS[:, 8:16])
            nc.vector.tensor_add(t2[:, :], accS[:, 16:24], accS[:, 24:32])
            nc.vector.tensor_add(t1[:, :], t1[:, :], t2[:, :])
            nc.vector.tensor_mul(t2[:, :], diagK[:, :], diagL[:, :])
            nc.vector.tensor_sub(t1[:, :], t1[:, :], t2[:, :])
            nc.vector.tensor_reduce(out_sb[:, 16:17], t1[:, :],
                                    axis=mybir.AxisListType.X, op=add)

            nc.gpsimd.dma_start(out=out_d[:, :], in_=out_sb[:, :])

    nc.compile()
    return nc


def _get_program():
    key = tuple(sorted(OPTS.items()))
    if key not in _cache:
        _cache[key] = _build_program()
    return _cache[key]


_EYE = None


def _eye_input():
    global _EYE
    if _EYE is None:
        _EYE = np.tile(np.eye(128, dtype=BF16), (NDEV, 1))
    return _EYE


def _f32_to_hilo(v64):
    hi = v64.astype(BF16)
    lo = (v64 - hi.astype(np.float64)).astype(BF16)
    return hi, lo


def prepare_xys(x8T, y8T):
    """[NDEV*128, 2*STRIP] fp8: per-core block = [x_strip^T | y_strip^T]."""
    XYS = np.empty((NDEV * 128, 2 * STRIP), dtype=FP8)
    for dev in range(NDEV):
        sl = slice(dev * STRIP, (dev + 1) * STRIP)
        XYS[dev * 128:(dev + 1) * 128, 0:STRIP] = x8T[:, sl]
        XYS[dev * 128:(dev + 1) * 128, STRIP:2 * STRIP] = y8T[:, sl]
    return XYS


def prepare_meta(xb, yb):
    """Packed per-core metadata [NDEV*160, 256] bf16; see meta_d layout.

    meta rows per core: [0:36] r2x hi, [36:72] r2x lo, [72:108] r2y hi,
    [108:144] r2y lo, [144:152] nsq hi, [152:160] nsq lo. The nsq flat
    order matches the SBUF [128, 2*NCHUNK] tile iteration
    (partition-major): element (p, off*NCHUNK + c) = -sq[strip][c*128+p].
    """
    META = np.empty((NDEV, 160, 256), dtype=BF16)
    for off, ab in ((0, xb), (1, yb)):
        af = ab.astype(np.float32)
        sq = (af * af).sum(axis=1, dtype=np.float64)      # [M] f64
        hi, lo = _f32_to_hilo(-sq / 2.0)                  # [M] bf16 each
        nhi, nlo = _f32_to_hilo(-sq)                      # [M] bf16 each
        r2o = off * 72
        for dev in range(NDEV):
            sl = slice(dev * STRIP, (dev + 1) * STRIP)
            blk = META[dev]
            for rows, vec in ((blk[r2o:r2o + 36], hi),
                              (blk[r2o + 36:r2o + 72], lo)):
                flat = rows.reshape(R2W)
                flat[0:M] = vec
                flat[M:R2W] = vec[sl]
            for rows, vec in ((blk[144:152], nhi), (blk[152:160], nlo)):
                rows.reshape(128, 2 * NCHUNK)[
                    :, off * NCHUNK:(off + 1) * NCHUNK] = \
                    vec[sl].reshape(NCHUNK, 128).T
    return META.reshape(NDEV * 160, 256)


def prepare_inputs(x, y):
    x8 = np.asarray(x, dtype=np.float32).astype(FP8)
    y8 = np.asarray(y, dtype=np.float32).astype(FP8)
    x8T = np.ascontiguousarray(x8.T)
    y8T = np.ascontiguousarray(y8.T)
    return {"xys": prepare_xys(x8T, y8T), "meta": prepare_meta(x8, y8)}


def combine(out_all):
    """Host-side unshard + closed-form diagonal. float64 combine.

    out_all: [NDEV, 128, 17] f32 device results.
    """
    out_all = np.asarray(out_all, dtype=np.float64)
    rK = np.ones(M, dtype=np.float64)
    rL = np.ones(M, dtype=np.float64)
    for dev in range(NDEV):
        sl = slice(dev * STRIP, (dev + 1) * STRIP)
        rK[sl] += out_all[dev, :, 0:8].T.reshape(STRIP)
        rL[sl] += out_all[dev, :, 8:16].T.reshape(STRIP)
    S_lk = float(M) + out_all[:, :, 16].sum()
    S_K = rK.sum()
    S_L = rL.sum()
    dotRR = (rK * rL).sum()
    hsic = (S_lk - 2.0 * dotRR / M + S_K * S_L / (float(M) ** 2)) \
        / float((M - 1) ** 2)
    return np.float32(hsic)


def _get_runner():
    """Build (once) a cached jitted SPMD runner over the 8 cores.

    Constant inputs (eye) and the dummy output operand buffers are
    device-resident and reused across calls; per-call work is only the
    3 data-dependent input transfers, dispatch, and one small fetch.
    """
    rkey = ("runner",) + tuple(sorted(OPTS.items()))
    if rkey in _cache:
        return _cache[rkey]
    import jax
    import numpy as _np
    from jax.sharding import Mesh, PartitionSpec, NamedSharding
    from jax.experimental.shard_map import shard_map
    from concourse import bass2jax as b2j
    import concourse.mybir as mybir

    b2j.install_neuronx_cc_hook()
    nc = _get_program()

    partition_name = (nc.partition_id_tensor.name
                      if nc.partition_id_tensor else None)
    in_names, out_names, out_avals, zero_outs = [], [], [], []
    for alloc in nc.m.functions[0].allocations:
        if not isinstance(alloc, mybir.MemoryLocationSet):
            continue
        name = alloc.memorylocations[0].name
        if alloc.kind == "ExternalInput":
            if name != partition_name:
                in_names.append(name)
        elif alloc.kind == "ExternalOutput":
            out_names.append(name)
            np_dt = mybir.dt.np(alloc.dtype)
            out_avals.append(jax.core.ShapedArray(
                tuple(alloc.tensor_shape), np_dt))
            zero_outs.append(_np.zeros(tuple(alloc.tensor_shape), np_dt))

    n_params = len(in_names)
    all_names = list(in_names) + list(out_names)
    if partition_name is not None:
        all_names = all_names + [partition_name]

    def _body(*args):
        operands = list(args)
        if partition_name is not None:
            operands.append(b2j.partition_id_tensor())
        outs = b2j._bass_exec_p.bind(
            *operands,
            out_avals=tuple(out_avals),
            in_names=tuple(all_names),
            out_names=tuple(out_names),
            lowering_input_output_aliases=(),
            sim_require_finite=True,
            sim_require_nnan=True,
            nc=nc,
        )
        return tuple(outs)

    devices = jax.devices()[:NDEV]
    mesh = Mesh(_np.asarray(devices), ("core",))
    sharding = NamedSharding(mesh, PartitionSpec("core"))
    n_ops = n_params + len(out_names)
    sharded = jax.jit(
        shard_map(_body, mesh=mesh,
                  in_specs=(PartitionSpec("core"),) * n_ops,
                  out_specs=(PartitionSpec("core"),) * len(out_names),
                  check_rep=False),
        keep_unused=True)

    # Device-resident constants: dummy output operands + the eye input.
    zero_dev = [
        jax.device_put(_np.zeros((NDEV * z.shape[0], *z.shape[1:]), z.dtype),
                       sharding)
        for z in zero_outs
    ]
    const_dev = {"eye": jax.device_put(_eye_input(), sharding)}

    _cache[rkey] = (sharded, in_names, out_names, out_avals, zero_dev,
                    const_dev, sharding)
    return _cache[rkey]


def run_device(arrays):
    """Run the SPMD program; returns out array [NDEV, 128, 17]."""
    import jax
    (sharded, in_names, out_names, out_avals, zero_dev, const_dev,
     sharding) = _get_runner()
    dev_in = [const_dev[nm] if nm in const_dev
              else jax.device_put(arrays[nm], sharding)
              for nm in in_names]
    out_arrs = sharded(*dev_in, *zero_dev)
    out = np.asarray(out_arrs[0])
    return out.reshape(NDEV, *out_avals[0].shape)


def kernel(x, y):
    import jax
    (sharded, in_names, out_names, out_avals, zero_dev, const_dev,
     sharding) = _get_runner()
    x8 = np.asarray(x, dtype=np.float32).astype(FP8)
    y8 = np.asarray(y, dtype=np.float32).astype(FP8)
    x8T = np.ascontiguousarray(x8.T)
    y8T = np.ascontiguousarray(y8.T)
    # Enqueue the big transfer first so it streams while the host
    # computes the metadata array.
    staged = {"xys": jax.device_put(prepare_xys(x8T, y8T), sharding)}
    staged["meta"] = jax.device_put(prepare_meta(x8, y8), sharding)
    dev_in = [const_dev[nm] if nm in const_dev else staged[nm]
              for nm in in_names]
    out_arrs = sharded(*dev_in, *zero_dev)
    out = np.asarray(out_arrs[0]).reshape(NDEV, *out_avals[0].shape)
    return combine(out)


def _timed_run(arrays, iters):
    """Min wall seconds for one dispatch of the current OPTS program."""
    import jax
    import time as _time
    (sharded, in_names, out_names, out_avals, zero_dev, const_dev,
     sharding) = _get_runner()
    dev_in = [const_dev[nm] if nm in const_dev
              else jax.device_put(arrays[nm], sharding)
              for nm in in_names]
    jax.block_until_ready(dev_in)
    best = float("inf")
    for i in range(iters + 1):
        t0 = _time.perf_counter()
        outs = sharded(*dev_in, *zero_dev)
        [np.asarray(o) for o in outs]
        dt = _time.perf_counter() - t0
        if i > 0:  # skip warm-up/compile call
            best = min(best, dt)
    return best


def time_on_hw(arrays, r_small=1, r_big=17, iters=8):
    """Estimate per-body HW time: (wall[R=r_big] - wall[R=r_small]) /
    (r_big - r_small), where R is the in-program body repeat count."""
    saved = OPTS["repeat"]
    walls = {}
    try:
        for r in (r_small, r_big):
            OPTS["repeat"] = r
            walls[r] = _timed_run(arrays, iters)
    finally:
        OPTS["repeat"] = saved
    per_body = (walls[r_big] - walls[r_small]) / (r_big - r_small)
    return per_body * 1e9, walls


# revision 17
# speedup vs baseline: 1.3596x; 1.0460x over previous
"""HSIC loss kernel for Trainium2, SPMD over 8 NeuronCores.

Math (reference): K = exp(-d2(x)), L = exp(-d2(y)),
  hsic = (sum(L*K) - 2*dot(rK,rL)/m + sum(K)*sum(L)/m^2) / (m-1)^2
where rK_i = sum_j K_ij (row sums; K, L symmetric).

Sharding: rows of the Gram matrices are split into 8 strips of 1024.
Each core receives ONLY its own strip of x and y (transposed, fp8
e4m3 - the inputs are exp() kernel arguments whose off-diagonal terms
are ~e-30, so 8-bit inputs are plenty; verified equal to the bf16
result at 3e-6 rel) plus a small packed metadata tensor; the full
x^T/y^T moving operands are assembled on-device with an AllGather
collective. This keeps host->device wire traffic at ~2.7 MB/call (vs
~39 MB if every core's full rotated copy were shipped), which
dominates end-to-end latency on the axon-tunneled PJRT transport
(~80 ms round-trip latency + ~120 MB/s effective stream rate).

Per core, the [1024, 8192] strips of K and L are computed fully fused
(never materialized in DRAM):
  PSUM = x_strip @ x_full^T  (fp8 matmul, D=128 contraction)
         + rank-2 correction folding in -sq_j/2 (bf16 hi/lo split)
  K    = ACT exp(2*PSUM - sq_i)  (per-partition bias, scale=2)
The diagonal needs exact treatment (off-diagonal entries are ~e-30;
the diagonal K_ii = 1 carries the whole answer). Because the strips
are gathered in natural order, the diagonal block position would be
core-dependent, which a static SPMD program cannot address. Instead
the main pass INCLUDES the (slightly inexact) diagonal, and a second
tiny pass recomputes the 8 diagonal [128,128] blocks bit-identically
from the local strip (same operand values, same accumulation order),
extracts their diagonals, and subtracts them from the row sums and
the K*L sum. The true diagonal (exp(0)=1) is re-added analytically
on the host - exact math, not an approximation.

Per-core output is a single [128, 17] f32 tensor: row sums of K and
L by chunk (diag excluded) and the K*L partial sum. Host combines in
float64.
"""

import numpy as np
import ml_dtypes

BF16 = ml_dtypes.bfloat16
FP8 = ml_dtypes.float8_e4m3

M = 8192
D = 128
NDEV = 8
STRIP = M // NDEV          # 1024 rows per core
NCHUNK = STRIP // 128      # 8 partition chunks per strip
SUPER = 2048               # ACT/PSUM super-tile width (4 PSUM banks)
NSUP = M // SUPER          # 4 j-supers
TS = 512                   # matmul free-dim tile (one PSUM bank)

R2W = M + STRIP            # 9216: full-M correction row + own-strip slice
NSLOT = NCHUNK * NSUP      # 32 accumulation slots

_cache = {}

OPTS = {"repeat": 1}


def _build_program():
    import concourse.bacc as bacc
    import concourse.mybir as mybir
    from concourse import tile

    f32 = mybir.dt.float32
    bf16 = mybir.dt.bfloat16
    f8 = mybir.dt.float8e4
    Exp = mybir.ActivationFunctionType.Exp
    mult = mybir.AluOpType.mult
    add = mybir.AluOpType.add

    nc = bacc.Bacc("TRN2", target_bir_lowering=False, debug=False,
                   num_devices=NDEV)

    # DRAM inputs (per-core values differ, same shapes: SPMD)
    # meta rows (x256 bf16): [0:36] r2x hi, [36:72] r2x lo, [72:108] r2y hi,
    # [108:144] r2y lo, [144:152] nsq hi, [152:160] nsq lo.
    xys_d = nc.dram_tensor("xys", [128, 2 * STRIP], f8, kind="ExternalInput")
    meta_d = nc.dram_tensor("meta", [160, 256], bf16, kind="ExternalInput")
    eye_d = nc.dram_tensor("eye", [128, 128], bf16, kind="ExternalInput")

    out_d = nc.dram_tensor("out", [128, 17], f32, kind="ExternalOutput")

    with tile.TileContext(nc) as tc:
        with (
            tc.tile_pool(name="dram", bufs=1, space="DRAM") as dram,
            tc.tile_pool(name="const", bufs=1) as cpool,
            tc.tile_pool(name="psum", bufs=2, space="PSUM") as pspool,
            tc.tile_pool(name="kl", bufs=2) as klpool,
            tc.tile_pool(name="scr", bufs=2) as scrpool,
        ):
            # --- AllGather the x/y strips into full moving operands ---
            cc_in = dram.tile([128, 2 * STRIP], f8)
            cc_out = dram.tile([NDEV * 128, 2 * STRIP], f8,
                               addr_space="Shared")
            nc.gpsimd.dma_start(out=cc_in[:, :], in_=xys_d[:, :])
            nc.gpsimd.collective_compute(
                "AllGather",
                mybir.AluOpType.bypass,
                replica_groups=[list(range(NDEV))],
                ins=[cc_in.opt()],
                outs=[cc_out.opt()],
            )

            xys = cpool.tile([128, 2 * STRIP], f8, tag="xys")
            r2x = cpool.tile([2, R2W], bf16, tag="r2x")
            r2y = cpool.tile([2, R2W], bf16, tag="r2y")
            nsq = cpool.tile([128, 2 * NCHUNK], f32, tag="nsq")
            nsqh = cpool.tile([128, 2 * NCHUNK], bf16, tag="nsqh")
            nsql = cpool.tile([128, 2 * NCHUNK], bf16, tag="nsql")
            nsqhf = cpool.tile([128, 2 * NCHUNK], f32, tag="nsqhf")
            nsqlf = cpool.tile([128, 2 * NCHUNK], f32, tag="nsqlf")
            eye = cpool.tile([128, 128], bf16, tag="eye")
            ones2 = cpool.tile([2, D], bf16, tag="ones2")
            xG = cpool.tile([128, M], f8, tag="xG")
            yG = cpool.tile([128, M], f8, tag="yG")
            accK = cpool.tile([128, NSLOT], f32, tag="accK")
            accL = cpool.tile([128, NSLOT], f32, tag="accL")
            accS = cpool.tile([128, NSLOT], f32, tag="accS")
            diagK = cpool.tile([128, NCHUNK], f32, tag="diagK")
            diagL = cpool.tile([128, NCHUNK], f32, tag="diagL")
            out_sb = cpool.tile([128, 17], f32, tag="out")
            t1 = cpool.tile([128, NCHUNK], f32, tag="t1")
            t2 = cpool.tile([128, NCHUNK], f32, tag="t2")
            u1 = cpool.tile([128, NCHUNK], f32, tag="u1")
            u2 = cpool.tile([128, NCHUNK], f32, tag="u2")

            nc.gpsimd.dma_start(out=xys[:, :], in_=xys_d[:, :])
            nc.gpsimd.dma_start(out=r2x[0:1, :], in_=meta_d[0:36, :])
            nc.gpsimd.dma_start(out=r2x[1:2, :], in_=meta_d[36:72, :])
            nc.gpsimd.dma_start(out=r2y[0:1, :], in_=meta_d[72:108, :])
            nc.gpsimd.dma_start(out=r2y[1:2, :], in_=meta_d[108:144, :])
            nc.gpsimd.dma_start(out=nsqh[:, :], in_=meta_d[144:152, :])
            nc.gpsimd.dma_start(out=nsql[:, :], in_=meta_d[152:160, :])
            nc.gpsimd.dma_start(out=eye[:, :], in_=eye_d[:, :])
            nc.vector.memset(ones2[:, :], 1.0)
            nc.vector.tensor_copy(nsqhf[:, :], nsqh[:, :])
            nc.vector.tensor_copy(nsqlf[:, :], nsql[:, :])
            nc.vector.tensor_add(nsq[:, :], nsqhf[:, :], nsqlf[:, :])

            # Gathered blocks -> SBUF full operands (block b at cols b*STRIP)
            for b in range(NDEV):
                rs = slice(b * 128, (b + 1) * 128)
                cs = slice(b * STRIP, (b + 1) * STRIP)
                nc.gpsimd.dma_start(out=xG[:, cs], in_=cc_out[rs, 0:STRIP])
                nc.gpsimd.dma_start(out=yG[:, cs],
                                    in_=cc_out[rs, STRIP:2 * STRIP])

            xTs = xys[:, 0:STRIP]
            yTs = xys[:, STRIP:2 * STRIP]

            # body emitted OPTS["repeat"] times (>1 only for HW timing:
            # outputs are identical per repeat, slope gives body time)
            for c in range(NCHUNK * OPTS["repeat"]):
                c = c % NCHUNK
                cs = slice(c * 128, (c + 1) * 128)
                for s in range(NSUP):
                    slot = s * NCHUNK + c       # acc layout: s-major
                    psK = pspool.tile([128, SUPER], f32, tag="ps")
                    psL = pspool.tile([128, SUPER], f32, tag="ps")
                    for t in range(NSUP):
                        jsl = slice(s * SUPER + t * TS, s * SUPER + (t + 1) * TS)
                        tsl = slice(t * TS, (t + 1) * TS)
                        nc.tensor.matmul(psK[:, tsl], lhsT=xTs[:, cs],
                                         rhs=xG[:, jsl], start=True, stop=False)
                    for t in range(NSUP):
                        jsl = slice(s * SUPER + t * TS, s * SUPER + (t + 1) * TS)
                        tsl = slice(t * TS, (t + 1) * TS)
                        nc.tensor.matmul(psK[:, tsl], lhsT=ones2[:, :],
                                         rhs=r2x[:, jsl], start=False, stop=True)
                    K_sb = klpool.tile([128, SUPER], bf16, tag="K")
                    nc.scalar.activation(K_sb[:, :], psK[:, :], Exp,
                                         bias=nsq[:, c:c + 1], scale=2.0,
                                         accum_out=accK[:, slot:slot + 1])

                    for t in range(NSUP):
                        jsl = slice(s * SUPER + t * TS, s * SUPER + (t + 1) * TS)
                        tsl = slice(t * TS, (t + 1) * TS)
                        nc.tensor.matmul(psL[:, tsl], lhsT=yTs[:, cs],
                                         rhs=yG[:, jsl], start=True, stop=False)
                    for t in range(NSUP):
                        jsl = slice(s * SUPER + t * TS, s * SUPER + (t + 1) * TS)
                        tsl = slice(t * TS, (t + 1) * TS)
                        nc.tensor.matmul(psL[:, tsl], lhsT=ones2[:, :],
                                         rhs=r2y[:, jsl], start=False, stop=True)
                    L_sb = klpool.tile([128, SUPER], bf16, tag="L")
                    nc.scalar.activation(L_sb[:, :], psL[:, :], Exp,
                                         bias=nsq[:, NCHUNK + c:NCHUNK + c + 1],
                                         scale=2.0,
                                         accum_out=accL[:, slot:slot + 1])

                    scr = scrpool.tile([128, SUPER], bf16, tag="scr")
                    nc.vector.scalar_tensor_tensor(
                        out=scr[:, :], in0=K_sb[:, :], scalar=1.0,
                        in1=L_sb[:, :], op0=mult, op1=mult,
                        accum_out=accS[:, slot:slot + 1])

            # --- pass B: recompute diagonal blocks bit-identically from the
            # local strip and extract their diagonals ---
            psDK = pspool.tile([128, SUPER], f32, tag="ps")
            psDL = pspool.tile([128, SUPER], f32, tag="ps")
            for c in range(NCHUNK):
                cs = slice(c * 128, (c + 1) * 128)
                nc.tensor.matmul(psDK[:, cs], lhsT=xTs[:, cs], rhs=xTs[:, cs],
                                 start=True, stop=False)
                nc.tensor.matmul(psDK[:, cs], lhsT=ones2[:, :],
                                 rhs=r2x[:, M + c * 128:M + (c + 1) * 128],
                                 start=False, stop=True)
                nc.tensor.matmul(psDL[:, cs], lhsT=yTs[:, cs], rhs=yTs[:, cs],
                                 start=True, stop=False)
                nc.tensor.matmul(psDL[:, cs], lhsT=ones2[:, :],
                                 rhs=r2y[:, M + c * 128:M + (c + 1) * 128],
                                 start=False, stop=True)
            KD = klpool.tile([128, SUPER], bf16, tag="K")
            LD = klpool.tile([128, SUPER], bf16, tag="L")
            for c in range(NCHUNK):
                cs = slice(c * 128, (c + 1) * 128)
                nc.scalar.activation(KD[:, cs], psDK[:, cs], Exp,
                                     bias=nsq[:, c:c + 1], scale=2.0)
                nc.scalar.activation(LD[:, cs], psDL[:, cs], Exp,
                                     bias=nsq[:, NCHUNK + c:NCHUNK + c + 1],
                                     scale=2.0)
            scrD = scrpool.tile([128, SUPER], bf16, tag="scr")
            for c in range(NCHUNK):
                cs = slice(c * 128, (c + 1) * 128)
                nc.vector.scalar_tensor_tensor(
                    out=scrD[:, cs], in0=KD[:, cs], scalar=1.0,
                    in1=eye[:, :], op0=mult, op1=mult,
                    accum_out=diagK[:, c:c + 1])
                nc.vector.scalar_tensor_tensor(
                    out=scrD[:, cs], in0=LD[:, cs], scalar=1.0,
                    in1=eye[:, :], op0=mult, op1=mult,
                    accum_out=diagL[:, c:c + 1])

            # --- final reductions: out[:, c] = sum_s acc[:, s*8+c] - diag ---
            nc.vector.tensor_add(t1[:, :], accK[:, 0:8], accK[:, 8:16])
            nc.vector.tensor_add(t2[:, :], accK[:, 16:24], accK[:, 24:32])
            nc.vector.tensor_add(t1[:, :], t1[:, :], t2[:, :])
            nc.vector.tensor_sub(out_sb[:, 0:8], t1[:, :], diagK[:, :])

            nc.vector.tensor_add(u1[:, :], accL[:, 0:8], accL[:, 8:16])
            nc.vector.tensor_add(u2[:, :], accL[:, 16:24], accL[:, 24:32])
            nc.vector.tensor_add(u1[:, :], u1[:, :], u2[:, :])
            nc.vector.tensor_sub(out_sb[:, 8:16], u1[:, :], diagL[:, :])

            nc.vector.tensor_add(t1[:, :], accS[:, 0:8], accS[:, 8:16])
            nc.vector.tensor_add(t2[:, :], accS[:, 16:24], accS[:, 24:32])
            nc.vector.tensor_add(t1[:, :], t1[:, :], t2[:, :])
            nc.vector.tensor_mul(t2[:, :], diagK[:, :], diagL[:, :])
            nc.vector.tensor_sub(t1[:, :], t1[:, :], t2[:, :])
            nc.vector.tensor_reduce(out_sb[:, 16:17], t1[:, :],
                                    axis=mybir.AxisListType.X, op=add)

            nc.gpsimd.dma_start(out=out_d[:, :], in_=out_sb[:, :])

    nc.compile()
    return nc


def _get_program():
    key = tuple(sorted(OPTS.items()))
    if key not in _cache:
        _cache[key] = _build_program()
    return _cache[key]


_EYE = None


def _eye_input():
    global _EYE
    if _EYE is None:
        _EYE = np.tile(np.eye(128, dtype=BF16), (NDEV, 1))
    return _EYE


def _f32_to_hilo(v64):
    hi = v64.astype(BF16)
    lo = (v64 - hi.astype(np.float64)).astype(BF16)
    return hi, lo


def prepare_xys(x8T, y8T):
    """[NDEV*128, 2*STRIP] fp8: per-core block = [x_strip^T | y_strip^T]."""
    XYS = np.empty((NDEV * 128, 2 * STRIP), dtype=FP8)
    for dev in range(NDEV):
        sl = slice(dev * STRIP, (dev + 1) * STRIP)
        XYS[dev * 128:(dev + 1) * 128, 0:STRIP] = x8T[:, sl]
        XYS[dev * 128:(dev + 1) * 128, STRIP:2 * STRIP] = y8T[:, sl]
    return XYS


def prepare_meta(xb, yb):
    """Packed per-core metadata [NDEV*160, 256] bf16; see meta_d layout.

    meta rows per core: [0:36] r2x hi, [36:72] r2x lo, [72:108] r2y hi,
    [108:144] r2y lo, [144:152] nsq hi, [152:160] nsq lo. The nsq flat
    order matches the SBUF [128, 2*NCHUNK] tile iteration
    (partition-major): element (p, off*NCHUNK + c) = -sq[strip][c*128+p].
    """
    META = np.empty((NDEV, 160, 256), dtype=BF16)
    for off, ab in ((0, xb), (1, yb)):
        af = ab.astype(np.float32)
        sq = (af * af).sum(axis=1, dtype=np.float64)      # [M] f64
        hi, lo = _f32_to_hilo(-sq / 2.0)                  # [M] bf16 each
        nhi, nlo = _f32_to_hilo(-sq)                      # [M] bf16 each
        r2o = off * 72
        for dev in range(NDEV):
            sl = slice(dev * STRIP, (dev + 1) * STRIP)
            blk = META[dev]
            for rows, vec in ((blk[r2o:r2o + 36], hi),
                              (blk[r2o + 36:r2o + 72], lo)):
                flat = rows.reshape(R2W)
                flat[0:M] = vec
                flat[M:R2W] = vec[sl]
            for rows, vec in ((blk[144:152], nhi), (blk[152:160], nlo)):
                rows.reshape(128, 2 * NCHUNK)[
                    :, off * NCHUNK:(off + 1) * NCHUNK] = \
                    vec[sl].reshape(NCHUNK, 128).T
    return META.reshape(NDEV * 160, 256)


def prepare_inputs(x, y):
    x8 = np.asarray(x, dtype=np.float32).astype(FP8)
    y8 = np.asarray(y, dtype=np.float32).astype(FP8)
    x8T = np.ascontiguousarray(x8.T)
    y8T = np.ascontiguousarray(y8.T)
    return {"xys": prepare_xys(x8T, y8T), "meta": prepare_meta(x8, y8)}


def combine(out_all):
    """Host-side unshard + closed-form diagonal. float64 combine.

    out_all: [NDEV, 128, 17] f32 device results.
    """
    out_all = np.asarray(out_all, dtype=np.float64)
    rK = np.ones(M, dtype=np.float64)
    rL = np.ones(M, dtype=np.float64)
    for dev in range(NDEV):
        sl = slice(dev * STRIP, (dev + 1) * STRIP)
        rK[sl] += out_all[dev, :, 0:8].T.reshape(STRIP)
        rL[sl] += out_all[dev, :, 8:16].T.reshape(STRIP)
    S_lk = float(M) + out_all[:, :, 16].sum()
    S_K = rK.sum()
    S_L = rL.sum()
    dotRR = (rK * rL).sum()
    hsic = (S_lk - 2.0 * dotRR / M + S_K * S_L / (float(M) ** 2)) \
        / float((M - 1) ** 2)
    return np.float32(hsic)


def _get_runner():
    """Build (once) a cached jitted SPMD runner over the 8 cores.

    Constant inputs (eye) and the dummy output operand buffers are
    device-resident and reused across calls; per-call work is only the
    3 data-dependent input transfers, dispatch, and one small fetch.
    """
    rkey = ("runner",) + tuple(sorted(OPTS.items()))
    if rkey in _cache:
        return _cache[rkey]
    import jax
    import numpy as _np
    from jax.sharding import Mesh, PartitionSpec, NamedSharding
    from jax.experimental.shard_map import shard_map
    from concourse import bass2jax as b2j
    import concourse.mybir as mybir

    b2j.install_neuronx_cc_hook()
    nc = _get_program()

    partition_name = (nc.partition_id_tensor.name
                      if nc.partition_id_tensor else None)
    in_names, out_names, out_avals, zero_outs = [], [], [], []
    for alloc in nc.m.functions[0].allocations:
        if not isinstance(alloc, mybir.MemoryLocationSet):
            continue
        name = alloc.memorylocations[0].name
        if alloc.kind == "ExternalInput":
            if name != partition_name:
                in_names.append(name)
        elif alloc.kind == "ExternalOutput":
            out_names.append(name)
            np_dt = mybir.dt.np(alloc.dtype)
            out_avals.append(jax.core.ShapedArray(
                tuple(alloc.tensor_shape), np_dt))
            zero_outs.append(_np.zeros(tuple(alloc.tensor_shape), np_dt))

    n_params = len(in_names)
    all_names = list(in_names) + list(out_names)
    if partition_name is not None:
        all_names = all_names + [partition_name]

    def _body(*args):
        operands = list(args)
        if partition_name is not None:
            operands.append(b2j.partition_id_tensor())
        outs = b2j._bass_exec_p.bind(
            *operands,
            out_avals=tuple(out_avals),
            in_names=tuple(all_names),
            out_names=tuple(out_names),
            lowering_input_output_aliases=(),
            sim_require_finite=True,
            sim_require_nnan=True,
            nc=nc,
        )
        return tuple(outs)

    devices = jax.devices()[:NDEV]
    mesh = Mesh(_np.asarray(devices), ("core",))
    sharding = NamedSharding(mesh, PartitionSpec("core"))
    n_ops = n_params + len(out_names)
    sharded = jax.jit(
        shard_map(_body, mesh=mesh,
                  in_specs=(PartitionSpec("core"),) * n_ops,
                  out_specs=(PartitionSpec("core"),) * len(out_names),
                  check_rep=False),
        keep_unused=True)

    # Device-resident constants: dummy output operands + the eye input.
    zero_dev = [
        jax.device_put(_np.zeros((NDEV * z.shape[0], *z.shape[1:]), z.dtype),
                       sharding)
        for z in zero_outs
    ]
    const_dev = {"eye": jax.device_put(_eye_input(), sharding)}

    _cache[rkey] = (sharded, in_names, out_names, out_avals, zero_dev,
                    const_dev, sharding)
    return _cache[rkey]


def run_device(arrays):
    """Run the SPMD program; returns out array [NDEV, 128, 17]."""
    import jax
    (sharded, in_names, out_names, out_avals, zero_dev, const_dev,
     sharding) = _get_runner()
    dev_in = [const_dev[nm] if nm in const_dev
              else jax.device_put(arrays[nm], sharding)
              for nm in in_names]
    out_arrs = sharded(*dev_in, *zero_dev)
    out = np.asarray(out_arrs[0])
    return out.reshape(NDEV, *out_avals[0].shape)


def kernel(x, y):
    import jax
    (sharded, in_names, out_names, out_avals, zero_dev, const_dev,
     sharding) = _get_runner()
    x8 = np.asarray(x, dtype=np.float32).astype(FP8)
    y8 = np.asarray(y, dtype=np.float32).astype(FP8)
    x8T = np.ascontiguousarray(x8.T)
    y8T = np.ascontiguousarray(y8.T)
    # Enqueue the big transfer first so it streams while the host
    # computes the metadata array.
    staged = {"xys": jax.device_put(prepare_xys(x8T, y8T), sharding)}
    staged["meta"] = jax.device_put(prepare_meta(x8, y8), sharding)
    dev_in = [const_dev[nm] if nm in const_dev else staged[nm]
              for nm in in_names]
    out_arrs = sharded(*dev_in, *zero_dev)
    out = np.asarray(out_arrs[0]).reshape(NDEV, *out_avals[0].shape)
    return combine(out)


def _timed_run(arrays, iters):
    """Min wall seconds for one dispatch of the current OPTS program."""
    import jax
    import time as _time
    (sharded, in_names, out_names, out_avals, zero_dev, const_dev,
     sharding) = _get_runner()
    dev_in = [const_dev[nm] if nm in const_dev
              else jax.device_put(arrays[nm], sharding)
              for nm in in_names]
    jax.block_until_ready(dev_in)
    best = float("inf")
    for i in range(iters + 1):
        t0 = _time.perf_counter()
        outs = sharded(*dev_in, *zero_dev)
        [np.asarray(o) for o in outs]
        dt = _time.perf_counter() - t0
        if i > 0:  # skip warm-up/compile call
            best = min(best, dt)
    return best


def time_on_hw(arrays, r_small=1, r_big=17, iters=8):
    """Estimate per-body HW time: (wall[R=r_big] - wall[R=r_small]) /
    (r_big - r_small), where R is the in-program body repeat count."""
    saved = OPTS["repeat"]
    walls = {}
    try:
        for r in (r_small, r_big):
            OPTS["repeat"] = r
            walls[r] = _timed_run(arrays, iters)
    finally:
        OPTS["repeat"] = saved
    per_body = (walls[r_big] - walls[r_small]) / (r_big - r_small)
    return per_body * 1e9, walls
